# revision 2
# baseline (speedup 1.0000x reference)
"""Trainium2 Bass kernel for the top-k ranking metric layer.

Computes, for each of 8192 users with 1000 candidates (1 positive + 999
negatives, channel 1 of a softmax pair):
  - in_top_k:  1.0 if the positive item ranks in the top 10 (after masking
               duplicate candidates to -inf), else 0.0
  - ndcg:      ln(2)/ln(rank+2) * in_top_k
  - weights:   1.0 unless all 999 negatives are duplicates

Key identity: with JAX's stable descending argsort, the rank of item 0 is
exactly  count_j(masked[j] > masked[0]).  No sort needed - it is a per-row
compare-and-count, which maps to a single fused DVE op per tile:
    cmp = (l[j] - v0) > d[j]*2^100 ;  rank = sum(cmp)    (accum_out)
where v0 = l[0] - d[0]*2^100.  The +-2^100 arithmetic is bit-exact
equivalent to the reference's big_neg masking for all tie cases because
fl(x +- 2^100) == +-2^100 exactly for |x| << 2^77.

Data-parallel across 8 NeuronCores: 1024 users per core.
"""

import numpy as np

_TRN_REPO = "/opt/trn_rl_repo"

NUM_CORES = 8
U = 8192                 # total users
ROW = 1000               # candidates per user
P = 128                  # SBUF partitions
U_CORE = U // NUM_CORES  # 1024 users per core
T = U_CORE // P          # 8 user-blocks per core
NPAIR = 2 * ROW          # interleaved (ch0, ch1) pairs per user
BIG = float(2.0 ** 100)
LN2 = float(np.log(2.0))
TOP_K = 10.0
DUP_ALL_NEG = 999.0 * BIG  # accumulated dup-sum value meaning "999 dups"

_NC = None


def _ensure_path():
    import sys
    try:
        import concourse  # noqa: F401
    except ImportError:
        sys.path.insert(0, _TRN_REPO)


def _build_nc():
    _ensure_path()
    from contextlib import ExitStack

    import concourse.tile as tile
    from concourse import bacc, mybir

    AF = mybir.ActivationFunctionType
    OP = mybir.AluOpType
    f32 = mybir.dt.float32
    i32 = mybir.dt.int32

    nc = bacc.Bacc(
        "TRN2", target_bir_lowering=False, debug=False, num_devices=NUM_CORES
    )
    ld = nc.dram_tensor("logits", [T, P, NPAIR], f32, kind="ExternalInput").ap()
    dd = nc.dram_tensor("dup", [T, P, ROW], i32, kind="ExternalInput").ap()
    outd = nc.dram_tensor("out", [P, 3 * T], f32, kind="ExternalOutput").ap()

    with tile.TileContext(nc) as tc, ExitStack() as ctx:
        lg = ctx.enter_context(tc.tile_pool(name="lg", bufs=3))
        dp = ctx.enter_context(tc.tile_pool(name="dp", bufs=3))
        ps = ctx.enter_context(tc.tile_pool(name="ps", bufs=3))
        cm = ctx.enter_context(tc.tile_pool(name="cm", bufs=2))
        sm = ctx.enter_context(tc.tile_pool(name="sm", bufs=4))
        st = ctx.enter_context(tc.tile_pool(name="st", bufs=1))

        cnt = st.tile([P, T], f32, tag="cnt")    # rank of item 0, per user
        dsm = st.tile([P, T], f32, tag="dsm")    # 2^100 * sum(dup), per user
        outt = st.tile([P, 3 * T], f32, tag="outt")

        for t in range(T):
            lt = lg.tile([P, NPAIR], f32, tag="lt")
            nc.sync.dma_start(lt[:], ld[t])
            dt_ = dp.tile([P, ROW], i32, tag="dt")
            nc.sync.dma_start(dt_[:], dd[t])

            # pos = dup * 2^100 (f32); accum gives 2^100 * row-sum(dup)
            pos = ps.tile([P, ROW], f32, tag="pos")
            nc.scalar.activation(
                pos[:], dt_[:], AF.Copy, scale=BIG, accum_out=dsm[:, t : t + 1]
            )

            l1 = lt[:, 1::2]  # channel-1 logits, strided view
            # v0 = l[0] - d[0]*2^100  (masked value of the positive item)
            v0 = sm.tile([P, 1], f32, tag="v0")
            nc.vector.tensor_tensor(v0[:], l1[:, 0:1], pos[:, 0:1], op=OP.subtract)
            # cmp[j] = (l[j] - v0) > d[j]*2^100 ; cnt = sum_j cmp[j]
            cmp = cm.tile([P, ROW], f32, tag="cmp")
            nc.vector.scalar_tensor_tensor(
                cmp[:],
                l1,
                v0[:],
                pos[:],
                op0=OP.subtract,
                op1=OP.is_gt,
                accum_out=cnt[:, t : t + 1],
            )

        # ---- finishing over [P, T] ----
        # in_top_k = rank < 10
        nc.vector.tensor_scalar(outt[:, 0:T], cnt[:], TOP_K, None, op0=OP.is_lt)
        # ndcg = ln2 / ln(rank + 2) * in_top_k
        two = st.tile([P, 1], f32, tag="two")
        nc.vector.memset(two[:], 2.0)
        lnp = st.tile([P, T], f32, tag="lnp")
        nc.scalar.activation(lnp[:], cnt[:], AF.Ln, bias=two[:])
        rcp = st.tile([P, T], f32, tag="rcp")
        nc.vector.reciprocal(rcp[:], lnp[:])
        nc.vector.scalar_tensor_tensor(
            outt[:, T : 2 * T],
            rcp[:],
            LN2,
            outt[:, 0:T],
            op0=OP.mult,
            op1=OP.mult,
        )
        # weights = (sum(dup) != 999)
        nc.vector.tensor_scalar(
            outt[:, 2 * T : 3 * T], dsm[:], DUP_ALL_NEG, None, op0=OP.not_equal
        )
        nc.sync.dma_start(outd, outt[:])

    nc.compile()
    return nc


def _get_nc():
    global _NC
    if _NC is None:
        _NC = _build_nc()
    return _NC


def _shard_inputs(logits, dup_mask):
    lg = np.ascontiguousarray(logits, dtype=np.float32).reshape(
        NUM_CORES, T, P, NPAIR
    )
    dm = np.ascontiguousarray(dup_mask, dtype=np.int32).reshape(NUM_CORES, T, P, ROW)
    return [{"logits": lg[c], "dup": dm[c]} for c in range(NUM_CORES)]


def _unshard_outputs(per_core_outs):
    # out[p, t] holds user t*128+p of the core (col-blocks: topk | ndcg | wts)
    full = np.stack(per_core_outs)  # [C, P, 3T]
    in_top_k = np.ascontiguousarray(
        full[:, :, 0:T].transpose(0, 2, 1).reshape(U), dtype=np.float32
    )
    ndcg = np.ascontiguousarray(
        full[:, :, T : 2 * T].transpose(0, 2, 1).reshape(U), dtype=np.float32
    )
    wts = np.ascontiguousarray(
        full[:, :, 2 * T : 3 * T].transpose(0, 2, 1).reshape(U), dtype=np.float32
    )
    return in_top_k, ndcg, wts


def _run(logits, dup_mask, trace=False, **kwargs):
    """Run on hardware; returns ((in_top_k, ndcg, weights), BassKernelResults)."""
    _ensure_path()
    from concourse.bass_utils import run_bass_kernel_spmd

    nc = _get_nc()
    in_maps = _shard_inputs(logits, dup_mask)
    res = run_bass_kernel_spmd(
        nc, in_maps, core_ids=list(range(NUM_CORES)), trace=trace, **kwargs
    )
    outs = [res.results[c]["out"] for c in range(NUM_CORES)]
    return _unshard_outputs(outs), res


def kernel(logits, dup_mask):
    (in_top_k, ndcg, wts), _ = _run(logits, dup_mask)
    return in_top_k, ndcg, wts
